# revision 26
# baseline (speedup 1.0000x reference)
"""CohortAwareBlock Trainium2 kernel.

Data-parallel over batch B=8 across 8 NeuronCores (one sample per core).
The cohort routing (gather of cohort_q_w by per-sample cohort id) happens on
the host while building each core's weight tensors, so the device kernel is a
plain attention block:

  per core (sample b):
    qT = (Wq_b * scale) @ x^T + bq_b*scale       [1024, N]   (e on partitions)
    kT = Wk @ x^T + bk                           [1024, N]
    v  = x @ Wv^T + bv                           [N, 1024]
    per head h (16 heads, hd=64):
      scoresT = [k, q] tiles via matmul(lhsT=kT_h chunk, rhs=qT_h); the two
                heads of a pair use K=64 row-groups 0:64 / 64:128 and write
                the two halves of a 2-bank PSUM tile, so they execute
                CONCURRENTLY on the PE (row tiling) and one ACT exp covers
                1024 columns.
      out2T   = [v_h | 1]^T-matmul: rows 0:64 = unnormalized attn-out^T,
                row 64 = softmax denominator (ones-column trick)
    attn-out^T lands in SBUF (attnT_sb); denominators DMA straight from PSUM
    to a tiny DRAM tile, bounce back as a partition-broadcast, and each
    (pair, q-half) normalizes in-place as soon as its attention finishes.
    proj: out = attnT^T @ projT + bp

All matmul inputs are fp16 (1 cycle/row on the PE); accumulation is fp32.
Initial DMAs are interleaved (xT chunk, wv chunk, ...) so the first v-gen
matmul starts as early as possible; proj weights load mid-attention.
"""

import numpy as np

import concourse.bass as bass
import concourse.bacc as bacc
import concourse.mybir as mybir
import concourse.tile as tile
from concourse.bass_utils import run_bass_kernel_spmd

P = 128
N = 1024            # sequence length
D = 1024            # model dim
H = 16              # heads
HD = 64             # head dim
NH = 2              # 512-wide halves of N
DC = D // P         # 8 contraction chunks
SCALE = HD ** -0.5
NCORES = 8

F32 = mybir.dt.float32
MM_DT = mybir.dt.float16


def _np_dt(dt):
    return mybir.dt.np(dt)


def build_nc(mm_dt=MM_DT):
    nc = bacc.Bacc(
        "TRN2",
        target_bir_lowering=False,
        debug=False,
        num_devices=NCORES,
    )

    # ---- external I/O (per-core shards, host-prepped layouts) ----
    xT = nc.dram_tensor("xT", [D, N], mm_dt, kind="ExternalInput")       # x^T
    wqk = nc.dram_tensor("wqk", [D, 2 * D], mm_dt, kind="ExternalInput")  # [D, e] cols: q(1024, scaled) then k(1024)
    bqk = nc.dram_tensor("bqk", [2 * D], F32, kind="ExternalInput")
    wv = nc.dram_tensor("wv", [D, D], mm_dt, kind="ExternalInput")        # [D, e_v]
    bv = nc.dram_tensor("bv", [D], F32, kind="ExternalInput")
    wp = nc.dram_tensor("wp", [D, D], mm_dt, kind="ExternalInput")        # proj_w^T: [e, f]
    bp = nc.dram_tensor("bp", [D], F32, kind="ExternalInput")
    out = nc.dram_tensor("out", [N, D], F32, kind="ExternalOutput")

    with tile.TileContext(nc) as tc:
        kernel_body(tc, xT, wqk, bqk, wv, bv, wp, bp, out, mm_dt)
    nc.compile()
    return nc


def kernel_body(tc, xT, wqk, bqk, wv, bv, wp, bp, out, mm_dt):
    nc = tc.nc
    EXP = mybir.ActivationFunctionType.Exp

    from contextlib import ExitStack

    with ExitStack() as ctx:
        resident = ctx.enter_context(tc.tile_pool(name="resident", bufs=1))
        dram = ctx.enter_context(tc.tile_pool(name="dram", bufs=1, space="DRAM"))
        # PSUM: 8 banks of [128, 2KB]. sc = 2x 2-bank tiles (paired heads),
        # o2 = 2x 1-bank, gen = 2x 1-bank (qk/v/proj accumulation chains).
        psum_sc = ctx.enter_context(tc.tile_pool(name="psum_sc", bufs=2, space="PSUM"))
        psum_o2 = ctx.enter_context(tc.tile_pool(name="psum_o2", bufs=2, space="PSUM"))
        psum_gen = ctx.enter_context(tc.tile_pool(name="psum_gen", bufs=2, space="PSUM"))

        wv_pool = ctx.enter_context(tc.tile_pool(name="wv_pool", bufs=2))

        # ---- resident tiles ----
        xT_sb = resident.tile([P, DC, N], mm_dt)
        bqk_sb = resident.tile([P, 16], F32)  # col t = bias for e-range t*128
        bv_rep = resident.tile([P, D], F32)
        bp_rep = resident.tile([P, D], F32)
        wp_sb = resident.tile([P, DC, D], mm_dt)
        # v_aug[p, kc, h, :]: cols 0:64 = v for head h at k-chunk kc, col 64 = 1.0
        v_aug = resident.tile([P, 8, H, HD + 1], mm_dt)
        # attn-out^T: tile co holds heads (2co, 2co+1) rows
        attnT_sb = resident.tile([P, DC, N], mm_dt)

        # ---- DRAM scratch: softmax denominators (partition-broadcast bounce)
        den_d = dram.tile([H, N], F32)

        # ---- startup loads, interleaved so v-gen's dc-0 matmul starts early
        xT_r = xT[:].rearrange("(dc p) n -> p dc n", p=P)
        wv_r = wv[:].rearrange("(dc p) e -> p dc e", p=P)
        wv_sb0 = wv_pool.tile([P, DC, 512], mm_dt, tag="wvc", name="wv_sb")
        wv_sb1 = wv_pool.tile([P, DC, 512], mm_dt, tag="wvc", name="wv_sb1")
        for dc in range(DC):
            nc.sync.dma_start(xT_sb[:, dc, :], xT_r[:, dc, :])
            nc.sync.dma_start(wv_sb0[:, dc, :], wv_r[:, dc, 0:512])
        nc.sync.dma_start(bv_rep[:], bv[None, :].to_broadcast([P, D]))
        nc.sync.dma_start(bqk_sb[:], bqk[:].rearrange("(t p) -> p t", p=P))
        nc.vector.memset(v_aug[:, :, :, HD : HD + 1], 1.0)

        wqk_r = wqk[:].rearrange("(dc p) e -> p dc e", p=P)
        wqk_pool = ctx.enter_context(tc.tile_pool(name="wqk_pool", bufs=6))
        wqk_tiles = {}

        def prefetch_wqk(co):
            if co >= DC:
                return
            tiles = []
            for qk in range(2):
                wc = wqk_pool.tile([P, DC, P], mm_dt, tag="wc", name="wc")
                e0 = qk * D + co * P
                nc.sync.dma_start(wc[:], wqk_r[:, :, e0 : e0 + P])
                tiles.append(wc)
            wqk_tiles[co] = tiles

        prefetch_wqk(0)
        for dc in range(DC):  # second wv half streams during eh=0 compute
            nc.sync.dma_start(wv_sb1[:, dc, :], wv_r[:, dc, 512:1024])

        # ================= Phase B: v generation (into resident v_aug) ======
        for eh in range(NH):
            wv_sb = wv_sb0 if eh == 0 else wv_sb1
            for nt2 in range(0, 8, 2):
                # two chains advance dc-interleaved: each freshly-arrived
                # (xT, wv) chunk feeds 2 matmuls, halving the early DMA
                # demand rate while the initial loads are still streaming
                pss = [
                    psum_gen.tile([P, 512], F32, tag="ps", name=f"ps{j}")
                    for j in range(2)
                ]
                for dc in range(DC):
                    for j in range(2):
                        nc.tensor.matmul(
                            pss[j][:],
                            lhsT=xT_sb[:, dc, (nt2 + j) * P : (nt2 + j + 1) * P],
                            rhs=wv_sb[:, dc, :],
                            start=(dc == 0),
                            stop=(dc == DC - 1),
                        )
                for j in range(2):
                    nc.vector.tensor_add(
                        v_aug[:, nt2 + j, eh * 8 : (eh + 1) * 8, 0:HD],
                        pss[j][:].rearrange("p (h d) -> p h d", d=HD),
                        bv_rep[:, eh * 512 : (eh + 1) * 512].rearrange(
                            "p (h d) -> p h d", d=HD
                        ),
                    )
            if eh == 0:
                prefetch_wqk(1)
                nc.sync.dma_start(bp_rep[:], bp[None, :].to_broadcast([P, D]))

        # ========== Fused phase A+C: qk gen + attention per head pair =======
        with tc.tile_pool(name="qp_pool", bufs=3) as qp_pool, tc.tile_pool(
            name="exp_pool", bufs=3
        ) as exp_pool, tc.tile_pool(name="att_ev", bufs=6) as att_ev, tc.tile_pool(
            name="norm_pool", bufs=4
        ) as norm_pool:
            pend = []  # lag-2 software pipeline queue for attn@v

            def emit_av(item):
                h, qh, exp_c, hh = item
                co = h // 2
                ps_o2 = psum_o2.tile([HD + 1, 512], F32, tag="o2", name="ps_o2")
                for kt in range(8):
                    nc.tensor.matmul(
                        ps_o2[:],
                        lhsT=v_aug[:, kt, h, :],
                        rhs=exp_c[:, kt, hh, :],
                        start=(kt == 0),
                        stop=(kt == 7),
                    )
                # denominator row: DVE evac (partition 64 -> 64) then DMA out
                dn = att_ev.tile([HD + 1, 512], F32, tag="dn", name="dn")
                nc.vector.tensor_copy(dn[HD : HD + 1, :], ps_o2[HD : HD + 1, :])
                nc.gpsimd.dma_start(
                    den_d[h : h + 1, qh * 512 : (qh + 1) * 512],
                    dn[HD : HD + 1, :],
                )
                if hh == 0:
                    # partitions line up: evac directly into the resident tile
                    nc.vector.tensor_copy(
                        attnT_sb[0:HD, co, qh * 512 : (qh + 1) * 512],
                        ps_o2[:HD, :],
                    )
                else:
                    # odd head: evac then DMA the partition move to rows 64:128
                    att = att_ev.tile([HD, 512], mm_dt, tag="att", name="att")
                    nc.vector.tensor_copy(att[:], ps_o2[:HD, :])
                    nc.gpsimd.dma_start(
                        attnT_sb[HD:P, co, qh * 512 : (qh + 1) * 512], att[:]
                    )
                    emit_norm(co, qh)

            def emit_norm(co, qh):
                # normalize pair co's attnT columns qh*512:+512 in-place
                sl = slice(qh * 512, (qh + 1) * 512)
                rc = norm_pool.tile([P, 512], F32, tag="rc", name="rc")
                nc.gpsimd.dma_start(
                    rc[0:HD, :], den_d[2 * co : 2 * co + 1, sl].to_broadcast([HD, 512])
                )
                nc.gpsimd.dma_start(
                    rc[HD:P, :],
                    den_d[2 * co + 1 : 2 * co + 2, sl].to_broadcast([HD, 512]),
                )
                nc.vector.reciprocal_approx_fast(rc[:], rc[:])
                nc.vector.tensor_mul(attnT_sb[:, co, sl], attnT_sb[:, co, sl], rc[:])

            for co in range(DC):  # head pair (2co, 2co+1)
                qp = qp_pool.tile([P, 2, N], mm_dt, tag="qp")  # [:,0]=q, [:,1]=k
                for qk in range(2):
                    wc = wqk_tiles[co][qk]
                    for nh in range(NH):
                        ps = psum_gen.tile([P, 512], F32, tag="ps")
                        for dc in range(DC):
                            nc.tensor.matmul(
                                ps[:],
                                lhsT=wc[:, dc, :],
                                rhs=xT_sb[:, dc, nh * 512 : (nh + 1) * 512],
                                start=(dc == 0),
                                stop=(dc == DC - 1),
                            )
                        nc.vector.tensor_scalar_add(
                            qp[:, qk, nh * 512 : (nh + 1) * 512],
                            ps[:],
                            bqk_sb[:, qk * 8 + co : qk * 8 + co + 1],
                        )
                prefetch_wqk(co + 2)
                if co == 1:  # proj weights: needed only at the tail
                    nc.sync.dma_start(
                        wp_sb[:], wp[:].rearrange("(co p) f -> p co f", p=P)
                    )

                # Attention: per kt, both heads' score matmuls go to disjoint
                # PE row-groups (K=64 at partition bases 0/64) and disjoint
                # PSUM banks -> they run concurrently. One ACT = exp over both.
                for qh in range(NH):
                    exp_c = exp_pool.tile(
                        [P, 8, 2, 512], mm_dt, tag="exps", name="exp_c"
                    )
                    for kt in range(8):
                        ps_sc = psum_sc.tile([P, 1024], F32, tag="sc", name="ps_sc")
                        for hh in range(2):
                            b0 = hh * HD
                            nc.tensor.matmul(
                                ps_sc[:, hh * 512 : (hh + 1) * 512],
                                lhsT=qp[b0 : b0 + HD, 1, kt * P : (kt + 1) * P],
                                rhs=qp[b0 : b0 + HD, 0, qh * 512 : (qh + 1) * 512],
                                start=True,
                                stop=True,
                            )
                        nc.scalar.activation(
                            exp_c[:, kt, :, :].rearrange("p a b -> p (a b)"),
                            ps_sc[:],
                            EXP,
                        )
                    for hh in range(2):
                        if len(pend) >= 2:
                            emit_av(pend.pop(0))
                        pend.append((2 * co + hh, qh, exp_c, hh))
            # ================= Phase D: projection =================
            # (first chains need only qh=0 norms, so they interleave with the
            #  flush of the last pended attn@v + the final qh=1 norms)
            with tc.tile_pool(name="out_ev", bufs=3) as out_ev:

                def emit_proj(nt, fh):
                    ps = psum_gen.tile([P, 512], F32, tag="ps")
                    for co in range(DC):
                        nc.tensor.matmul(
                            ps[:],
                            lhsT=attnT_sb[:, co, nt * P : (nt + 1) * P],
                            rhs=wp_sb[:, co, fh * 512 : (fh + 1) * 512],
                            start=(co == 0),
                            stop=(co == DC - 1),
                        )
                    ev = out_ev.tile([P, 512], F32, tag="oev")
                    nc.vector.tensor_add(
                        ev[:], ps[:], bp_rep[:, fh * 512 : (fh + 1) * 512]
                    )
                    nc.sync.dma_start(
                        out[nt * P : (nt + 1) * P, fh * 512 : (fh + 1) * 512],
                        ev[:],
                    )

                flush = list(pend)
                pend = []
                done = []
                for i, item in enumerate(flush):
                    emit_av(item)
                    emit_proj(0, i)  # nt=0 needs only qh=0 norms (all done)
                    done.append((0, i))
                for nt in range(8):
                    for fh in range(NH):
                        if (nt, fh) not in done:
                            emit_proj(nt, fh)


def make_in_maps(x, c, kv_w, kv_b, shared_q_w, shared_q_b, cohort_q_w, cohort_q_b,
                 proj_w, proj_b, mm_dt=MM_DT):
    np_dt = _np_dt(mm_dt)
    f32 = np.float32
    x = np.asarray(x, dtype=f32)
    c = np.asarray(c).astype(np.int64)
    kv_w = np.asarray(kv_w, dtype=f32)
    kv_b = np.asarray(kv_b, dtype=f32)
    shared_q_w = np.asarray(shared_q_w, dtype=f32)
    shared_q_b = np.asarray(shared_q_b, dtype=f32)
    cohort_q_w = np.asarray(cohort_q_w, dtype=f32)
    cohort_q_b = np.asarray(cohort_q_b, dtype=f32)
    proj_w = np.asarray(proj_w, dtype=f32)
    proj_b = np.asarray(proj_b, dtype=f32)

    wk = kv_w[:D]       # [1024, D]
    wv_ = kv_w[D:]      # [1024, D]
    bk = kv_b[:D]
    bv_ = kv_b[D:]

    wvT = np.ascontiguousarray(wv_.T).astype(np_dt)          # [D, e_v]
    wpT = np.ascontiguousarray(proj_w.T).astype(np_dt)       # [e, f]
    bp_ = np.ascontiguousarray(proj_b)

    in_maps = []
    for b in range(x.shape[0]):
        wq = np.concatenate([shared_q_w, cohort_q_w[c[b]]], axis=0) * SCALE
        bq = np.concatenate([shared_q_b, cohort_q_b[c[b]]], axis=0) * SCALE
        wqk = np.ascontiguousarray(np.concatenate([wq, wk], axis=0).T).astype(np_dt)
        bqk = np.concatenate([bq, bk]).astype(f32)
        in_maps.append(
            {
                "xT": np.ascontiguousarray(x[b].T).astype(np_dt),
                "wqk": wqk,
                "bqk": bqk,
                "wv": wvT,
                "bv": bv_,
                "wp": wpT,
                "bp": bp_,
            }
        )
    return in_maps


_NC_CACHE = {}


def kernel(**inputs) -> np.ndarray:
    in_maps = make_in_maps(**inputs)
    if MM_DT not in _NC_CACHE:
        _NC_CACHE[MM_DT] = build_nc(MM_DT)
    nc = _NC_CACHE[MM_DT]
    res = run_bass_kernel_spmd(nc, in_maps, core_ids=list(range(NCORES)))
    out = np.stack([res.results[i]["out"] for i in range(NCORES)], axis=0)
    return out.astype(np.float32)
